# revision 3
# baseline (speedup 1.0000x reference)
"""Causal scaled-dot-product attention on 8 Trainium2 NeuronCores.

Problem: B=2, H=16, S=2048, D=64, fp32, causal mask.
Sharding: batch*heads (32) split 4-per-core across 8 cores; no collectives.

Per-core bass/Tile kernel, heads in pairs (head A on SBUF partitions 0-63,
head B on 64-127 so their K=64 matmuls occupy disjoint PE row groups and
run concurrently):

Phase 1 (per k-chunk row ci, both heads):
  - S^T[k, q] = (K^T)^T @ Q^T on PE (fp16), full causal span
    q in [128ci, S), in pieces of <=PIECE_W columns (PSUM).
  - exp(scale * S^T) -> persistent SBUF fp16, causally packed. Pieces are
    load-balanced across TWO engines:
      * ScalarE ACT: exact exp via activation LUT
      * VectorE DVE: Schraudolph fp16 exp - one tensor_scalar
        (i16 = rint(s*SCALE*1024/ln2 + 15300); bitcast int16->fp16
        IS 2^(x*log2 e) to ~1.8% rms).  The softmax denominator uses the
        same approximated values, so the bias largely cancels.
  - Diagonal 128x128 tile: GPSIMD affine_select zeroes P^T where k > q.

Phase 2 (interleaved, q-tile qt = ci just produced):
  - O[q, 0:64] and l=O[q, 64] accumulated in PSUM over chunks ci<=qt:
    matmul(P^T chunk stationary, V_aug moving), V_aug = [V | 1].
    Four consecutive q-tiles of one head share one PSUM bank tile
    [128, 4, 65]; after the 4th chain:
      * DVE reciprocal of the 4 l-columns -> rr[128, 4]
      * DVE grouped copy PSUM -> SBUF   (one instr per 4 chains)
      * GPSIMD broadcast multiply (stride-0 AP on rr) normalizes in SBUF
      * one contiguous DMA [128, 260] f32 -> DRAM
Host packs Q/K transposed (head pairs stacked on partitions) and V chunked
with a ones column, fp16; host unpack drops the l columns + transposes.
"""

import sys
import numpy as np
from contextlib import ExitStack

B, H, S, D = 2, 16, 2048, 64
N_CORES = 8
HEADS_PER_CORE = (B * H) // N_CORES  # 4
CH = 128             # k-chunk (partition tile)
PIECE_W = 512        # S^T piece width per head (paired -> FD=1024, 2 banks)
GRP = 4              # phase-2 q-tiles per PSUM bank group
SCALE = 1.0 / np.sqrt(D)
MM_DTYPE = "float16"
# Schraudolph constants (fp16 bit trick), applied to RAW scores s:
# i16 = rint(s * SCH_A + SCH_B);  fp16frombits(i16) ~ exp(s * SCALE)
SCH_A = float(1024.0 / np.log(2.0) / np.sqrt(D))
SCH_B = 15300.0          # 15*1024 - 60 (delta tuned for min bias)
DVE_FRAC_CAP = 0.50      # max fraction of exp columns on the DVE path

_NP_MM = {"float16": np.float16, "float32": np.float32}

for _p in ("/opt/trn_rl_repo", "/opt/pypackages"):
    if _p not in sys.path:
        sys.path.append(_p)


def _row_off(ci, s_len):
    # packed column offset of causal row ci: sum_{j<ci} (s_len - 128*j)
    return s_len * ci - CH * (ci * (ci - 1)) // 2


def _build_program(n_heads, s_len, piece_w=PIECE_W, mm_dtype=MM_DTYPE):
    import concourse.bass as bass  # noqa: F401
    import concourse.bacc as bacc
    import concourse.tile as tile
    from concourse import mybir
    from concourse.alu_op_type import AluOpType
    from concourse.bass_types import AP

    f32 = mybir.dt.float32
    i16 = mybir.dt.int16
    mmdt = getattr(mybir.dt, mm_dtype)
    n_chunks = s_len // CH
    n_pairs = (n_heads + 1) // 2
    DP1 = D + 1
    pt_len = _row_off(n_chunks, s_len)  # packed P^T length per head
    n_grps = n_chunks // GRP

    nc = bacc.Bacc(
        "TRN2",
        target_bir_lowering=False,
        debug=False,
        num_devices=N_CORES,
    )

    qk_d = nc.dram_tensor("qk", [128, n_pairs, 2, s_len], mmdt, kind="ExternalInput").ap()
    v_d = nc.dram_tensor("v", [128, n_heads, n_chunks, DP1], mmdt, kind="ExternalInput").ap()
    o_d = nc.dram_tensor("o", [n_heads, 128, n_grps, GRP * DP1], f32, kind="ExternalOutput").ap()

    with tile.TileContext(nc) as tc, ExitStack() as ctx:
        const = ctx.enter_context(tc.tile_pool(name="const", bufs=1))
        sb_pt = ctx.enter_context(tc.tile_pool(name="ptp", bufs=2))
        sb_o = ctx.enter_context(tc.tile_pool(name="osb", bufs=6))
        sb_r = ctx.enter_context(tc.tile_pool(name="rsb", bufs=6))
        # PSUM: 2 x S-tile (2 banks each) + 4 x phase-2 group tile (1 bank)
        ps_s = ctx.enter_context(tc.tile_pool(name="pss", bufs=2, space="PSUM"))
        ps_o = ctx.enter_context(tc.tile_pool(name="pso", bufs=4, space="PSUM"))

        qk = const.tile([128, n_pairs, 2, s_len], mmdt)
        v = const.tile([128, n_heads, n_chunks, DP1], mmdt)
        for pair in range(n_pairs):
            nc.sync.dma_start(out=qk[:, pair], in_=qk_d[:, pair])
        for hh in range(n_heads):
            nc.sync.dma_start(out=v[:, hh], in_=v_d[:, hh])

        # engine load accounting (ns) for the exp piece scheduler
        load = {"act": 0.0, "dve": 0.0}
        elems = {"act": 1.0, "dve": 0.0}

        def exp_piece(pt_pair, nh, ro, poff, w, st):
            """exp of one S piece [128, nh, w] PSUM -> pt fp16, on the
            engine with the least projected load."""
            fd = nh * w
            c_act = (fd + 172) / 1.2
            c_dve = (fd + 120) / 0.96
            use_dve = (load["dve"] + c_dve < load["act"] + c_act) and (
                elems["dve"] / (elems["act"] + elems["dve"]) < DVE_FRAC_CAP
            )
            dst = pt_pair[:, 0:nh, ro + poff:ro + poff + w]
            if use_dve:
                load["dve"] += c_dve
                elems["dve"] += fd
                nc.vector.tensor_scalar(
                    dst.bitcast(i16), st[:, 0:nh, 0:w],
                    SCH_A, SCH_B, AluOpType.mult, AluOpType.add,
                )
            else:
                load["act"] += c_act
                elems["act"] += fd
                nc.scalar.activation(
                    dst, st[:, 0:nh, 0:w],
                    mybir.ActivationFunctionType.Exp, scale=float(SCALE),
                )

        def ph1_row(pair, heads, ci, pt_pair):
            sp0 = CH * ci
            span = s_len - sp0
            ro = _row_off(ci, s_len)
            for poff in range(0, span, piece_w):
                w = min(piece_w, span - poff)
                st = ps_s.tile([128, 2, piece_w], f32, tag="st")
                for idx, hh in enumerate(heads):
                    bp = 64 * (hh % 2)
                    nc.tensor.matmul(
                        st[:, idx, 0:w],
                        qk[bp:bp + 64, pair, 1, sp0:sp0 + CH],
                        qk[bp:bp + 64, pair, 0, sp0 + poff:sp0 + poff + w],
                        start=True,
                        stop=True,
                    )
                exp_piece(pt_pair, len(heads), ro, poff, w, st)
                if poff == 0:
                    for idx in range(len(heads)):
                        nc.gpsimd.affine_select(
                            out=pt_pair[:, idx, ro:ro + CH],
                            in_=pt_pair[:, idx, ro:ro + CH],
                            compare_op=mybir.AluOpType.is_ge,
                            fill=0.0,
                            base=0,
                            pattern=[[1, CH]],
                            channel_multiplier=-1,
                        )

        group_tiles = {}

        def ph2_chain(hh, idx, qt, pt_pair):
            """accumulate O[q-tile qt] into its group tile; on group end,
            normalize (DVE recip + grouped copy, GPSIMD bcast mult) + DMA."""
            g, gi = divmod(qt, GRP)
            if gi == 0:
                group_tiles[hh] = ps_o.tile([128, GRP, DP1], f32, tag="op",
                                            name=f"op{hh}g{g}")
            op = group_tiles[hh]
            for ci in range(qt + 1):
                sl = _row_off(ci, s_len) + CH * (qt - ci)
                nc.tensor.matmul(
                    op[:, gi, :],
                    pt_pair[:, idx, sl:sl + CH],
                    v[:, hh, ci, :],
                    start=(ci == 0),
                    stop=(ci == qt),
                )
            if gi == GRP - 1:
                rr = sb_r.tile([128, GRP], f32, tag="rr")
                nc.vector.reciprocal(rr, op[:, :, D])
                load["dve"] += (GRP + 120) / 0.96
                o_sb = sb_o.tile([128, GRP, DP1], f32, tag="osb",
                                 name=f"os{hh}g{g}")
                nc.vector.tensor_copy(o_sb, op)
                load["dve"] += (GRP * DP1 + 120) / 0.96
                rr_ap = rr[:, :]
                rb = AP(rr_ap.tensor, rr_ap.offset, list(rr_ap.ap) + [[0, D]])
                nc.gpsimd.tensor_tensor(
                    o_sb[:, :, 0:D], o_sb[:, :, 0:D], rb, AluOpType.mult
                )
                nc.sync.dma_start(
                    out=o_d[hh][:, g],
                    in_=o_sb.rearrange("p g c -> p (g c)"),
                )

        LAG = 2
        pending = []

        def emit_pending(k):
            for _ in range(min(k, len(pending))):
                pending.pop(0)()

        pair_heads = {p: [hh for hh in (2 * p, 2 * p + 1) if hh < n_heads]
                      for p in range(n_pairs)}
        pts = {p: sb_pt.tile([128, 2, pt_len], mmdt, tag="ptfull", name=f"ptp{p}")
               for p in range(n_pairs)}

        chains_per_row = sum(len(v) for v in pair_heads.values())
        for ci in range(n_chunks):
            for pair in range(n_pairs):
                ph1_row(pair, pair_heads[pair], ci, pts[pair])
                for idx, hh in enumerate(pair_heads[pair]):
                    pending.append(
                        (lambda h=hh, i=idx, q=ci, p=pts[pair]:
                         ph2_chain(h, i, q, p))
                    )
            emit_pending(len(pending) - LAG * chains_per_row)
        emit_pending(len(pending))

    nc.compile()
    return nc


_PROGRAM_CACHE = {}


def _get_program(n_heads=HEADS_PER_CORE, s_len=S, piece_w=PIECE_W, mm_dtype=MM_DTYPE):
    key = (n_heads, s_len, piece_w, mm_dtype)
    if key not in _PROGRAM_CACHE:
        _PROGRAM_CACHE[key] = _build_program(n_heads, s_len, piece_w, mm_dtype)
    return _PROGRAM_CACHE[key]


def _pack_core(Qf, Kf, Vf, heads, s_len=S, mm_dtype=MM_DTYPE):
    """Build the per-core input dict. Qf/Kf/Vf: [B*H, S, D] float32."""
    dt_np = _NP_MM[mm_dtype]
    n_heads = len(heads)
    n_pairs = (n_heads + 1) // 2
    n_chunks = s_len // CH
    qk = np.zeros((128, n_pairs, 2, s_len), dt_np)
    v = np.ones((128, n_heads, n_chunks, D + 1), dt_np)
    for i, hf in enumerate(heads):
        pair, side = divmod(i, 2)
        bp = 64 * side
        qk[bp:bp + 64, pair, 0] = Qf[hf].T
        qk[bp:bp + 64, pair, 1] = Kf[hf].T
        v[:, i, :, :D] = Vf[hf].reshape(n_chunks, CH, D).transpose(1, 0, 2)
    return {"qk": qk, "v": v}


def _unpack_core(o_np, s_len=S):
    """o_np: [n_heads, 128, n_grps, GRP*(D+1)] -> [n_heads, S, D]."""
    n_heads = o_np.shape[0]
    n_chunks = s_len // CH
    o = o_np.reshape(n_heads, 128, n_chunks, D + 1)[:, :, :, :D]
    return o.transpose(0, 2, 1, 3).reshape(n_heads, s_len, D)


def kernel(Q, K, V, mask):
    Q = np.asarray(Q, np.float32)
    K = np.asarray(K, np.float32)
    V = np.asarray(V, np.float32)
    mask = np.asarray(mask)

    if not np.array_equal(mask, np.tril(np.ones((S, S), dtype=bool))):
        scores = np.einsum("bhqd,bhkd->bhqk", Q, K) * SCALE
        scores = np.where(mask, scores, -np.inf)
        scores -= scores.max(-1, keepdims=True)
        p = np.exp(scores)
        p /= p.sum(-1, keepdims=True)
        return np.einsum("bhqk,bhkd->bhqd", p, V).astype(np.float32)

    from concourse.bass_utils import run_bass_kernel_spmd

    Qf = Q.reshape(B * H, S, D)
    Kf = K.reshape(B * H, S, D)
    Vf = V.reshape(B * H, S, D)

    nc = _get_program()
    in_maps = [
        _pack_core(Qf, Kf, Vf, list(range(c * HEADS_PER_CORE, (c + 1) * HEADS_PER_CORE)))
        for c in range(N_CORES)
    ]
    res = run_bass_kernel_spmd(nc, in_maps, core_ids=list(range(N_CORES)))
    out = np.empty((B * H, S, D), np.float32)
    for c in range(N_CORES):
        out[c * HEADS_PER_CORE:(c + 1) * HEADS_PER_CORE] = _unpack_core(res.results[c]["o"])
    return out.reshape(B, H, S, D)
